# revision 6
# baseline (speedup 1.0000x reference)
"""Trainium2 Bass kernel for nn_CopyGenerator (scatter_memory).

Computation (see the reference):
  out_tgt = log_softmax(hidden @ W.T + b)                    [T,B,VT]
  gate1m  = 1 - sigmoid(dec @ Wc.T + bc)                     [T,B]
  ext[t,b,v] = gate1m[t,b] * sum_s attn[t,b,s]*(idx[s,b]==v), idx==UNK masked
  out_ext = log(clip(ext, 1e-3, 1-1e-3))                     [T,B,VE]
  out = concat([out_tgt, out_ext], -1)

Sharding (8 cores):
  - Big matmul + log_softmax: column-parallel over tgt vocab (each core owns a
    4000-wide W slice, SBUF-resident bf16; all 3200 rows). The softmax
    denominator needs the full-vocab sum -> per-chunk partial row sums are
    AllReduced across cores (5 tiny pipelined collectives).
  - Copy-gate + scatter-add over ext vocab: data-parallel over batch (8 batch
    elements per core). The scatter-add is aw.T @ onehot(idx) on the tensor
    engine (exact, handles duplicate indices); attn is fed as a bf16 hi/lo
    pair so the ext sums are fp32-accurate.
"""

import sys

if "/opt/trn_rl_repo" not in sys.path:
    sys.path.insert(0, "/opt/trn_rl_repo")

from contextlib import ExitStack

import ml_dtypes
import numpy as np

import concourse.bass as bass
import concourse.mybir as mybir
import concourse.tile as tile
from concourse import bacc
from concourse.bass_utils import run_bass_kernel_spmd

F32 = mybir.dt.float32
BF16 = mybir.dt.bfloat16
AF = mybir.ActivationFunctionType
OP = mybir.AluOpType

T, B, S, H = 50, 64, 100, 512
VT, VE = 32000, 5000
N_CORES = 8
VL = VT // N_CORES       # 4000 vocab cols per core
BL = B // N_CORES        # 8 batch per core (ext part)
R = T * B                # 3200 rows
RL = T * BL              # 400 rows (ext part)
KT = H // 128            # 4 k-tiles
MT = R // 128            # 25 m-tiles
CH = 5                   # m-tiles per lse chunk
NCH = MT // CH           # 5 chunks
NW = 500                 # main n-block width (<=512 f32 per psum bank)
NQ = 4                   # psum pairs per m-tile (2 n-blocks each)
EXT_N = 500
EXT_NB = VE // EXT_N     # 10

LOG_LO = float(np.log(0.001))
LOG_HI = float(np.log(1.0 - 0.001))

_CACHE = {}


def _dedupe_act_table_loads(nc):
    """Collapse activation-table thrash: point every load at a table that
    serves its following activations when one exists, then drop loads that
    re-load the already-loaded table. Saves ~1.8us per removed load on ACT."""
    from concourse.hw_specs import get_activation_tables
    tables = list(get_activation_tables(nc.m.arch).items())
    for blk in nc.m.functions[0].blocks:
        insts = blk.instructions
        loads = [(i, inst) for i, inst in enumerate(insts)
                 if isinstance(inst, mybir.InstLoadActFuncSet)]
        if not loads:
            continue
        for li, (pos, inst) in enumerate(loads):
            end = loads[li + 1][0] if li + 1 < len(loads) else len(insts)
            funcs = {s.func for s in insts[pos:end]
                     if isinstance(s, mybir.InstActivation)}
            if not funcs:
                continue
            want = funcs | {AF.Exp, AF.Ln, AF.Identity, AF.Copy}
            pick = None
            for tid, (name, fs) in enumerate(tables):
                if want <= fs:
                    pick = tid
                    break
            if pick is None:
                for tid, (name, fs) in enumerate(tables):
                    if funcs <= fs:
                        pick = tid
                        break
            if pick is not None:
                inst.act_func_set_id = pick
        cur = None
        to_drop = []
        for pos, inst in loads:
            if cur is not None and inst.act_func_set_id == cur:
                si = inst.sync_info
                clean = si is None or (not si.on_wait and not si.on_update)
                if clean:
                    to_drop.append(inst)
                    continue
            cur = inst.act_func_set_id
        for inst in to_drop:
            insts.remove(inst)


def _build(with_bias):
    nc = bacc.Bacc("TRN2", target_bir_lowering=False, debug=False,
                   num_devices=N_CORES)

    hT = [nc.dram_tensor(f"hT{k}", [128, R], BF16, kind="ExternalInput").ap()
          for k in range(KT)]
    wT = [nc.dram_tensor(f"wT{k}", [128, VL], BF16, kind="ExternalInput").ap()
          for k in range(KT)]
    if with_bias:
        brow = nc.dram_tensor("brow", [1, VL], BF16, kind="ExternalInput").ap()
    dT = nc.dram_tensor("dT", [KT, 128, RL], F32, kind="ExternalInput").ap()
    wcT = nc.dram_tensor("wcT", [KT, 128, 1], F32, kind="ExternalInput").ap()
    bc_t = nc.dram_tensor("bc", [1, 1], F32, kind="ExternalInput").ap()
    # attn hi/lo bf16 split: [2, S, BL*T]
    attnT = nc.dram_tensor("attnT", [2, S, BL * T], BF16, kind="ExternalInput").ap()
    idx_t = nc.dram_tensor("idx", [S, BL], F32, kind="ExternalInput").ap()

    out_tgt = nc.dram_tensor("out_tgt", [R, VL], F32, kind="ExternalOutput").ap()
    out_ext = nc.dram_tensor("out_ext", [BL, T, VE], F32, kind="ExternalOutput").ap()

    cc_in = [nc.dram_tensor(f"cc_in{g}", [128, CH], F32).ap() for g in range(NCH)]
    cc_out = [nc.dram_tensor(f"cc_out{g}", [128, CH], F32, addr_space="Shared").ap()
              for g in range(NCH)]

    core_ids = list(range(N_CORES))

    with tile.TileContext(nc) as tc, ExitStack() as ctx:
        const = ctx.enter_context(tc.tile_pool(name="const", bufs=1))
        xpool = ctx.enter_context(tc.tile_pool(name="x", bufs=8))
        epool = ctx.enter_context(tc.tile_pool(name="E", bufs=1))
        outpool = ctx.enter_context(tc.tile_pool(name="out", bufs=3))
        statpool = ctx.enter_context(tc.tile_pool(name="stat", bufs=2))
        ohpool = ctx.enter_context(tc.tile_pool(name="oh", bufs=2))
        extstage = ctx.enter_context(tc.tile_pool(name="exts", bufs=4))
        ps_main = ctx.enter_context(tc.tile_pool(name="psm", bufs=3, space="PSUM"))
        ps_ext = ctx.enter_context(tc.tile_pool(name="pse", bufs=2, space="PSUM"))

        # ---- persistent SBUF loads (per-k tiles so PE can start early) ----
        hT_sb = [const.tile([128, R], BF16, name=f"hts{k}") for k in range(KT)]
        wT_sb = [const.tile([128, VL], BF16, name=f"wts{k}") for k in range(KT)]
        for k in range(KT):
            nc.sync.dma_start(wT_sb[k][:], wT[k])
            nc.sync.dma_start(hT_sb[k][:], hT[k])
        if with_bias:
            b_sb = const.tile([1, VL], BF16)
            nc.sync.dma_start(b_sb[:], brow[:])
            ones_sb = const.tile([1, 128], BF16)
            nc.vector.memset(ones_sb[:], 1.0)
        dT_sb = const.tile([128, KT * RL], F32)
        for k in range(KT):
            nc.sync.dma_start(dT_sb[:, k * RL:(k + 1) * RL], dT[k])
        wcT_sb = const.tile([128, KT], F32)
        for k in range(KT):
            nc.sync.dma_start(wcT_sb[:, k:k + 1], wcT[k])
        bc_sb = const.tile([1, 1], F32)
        nc.sync.dma_start(bc_sb[:], bc_t[:])
        ones50 = const.tile([1, 64], F32)
        nc.vector.memset(ones50[:], 1.0)
        attnT_sb = const.tile([S, 2 * BL * T], BF16)
        nc.sync.dma_start(attnT_sb[:, :BL * T], attnT[0])
        nc.sync.dma_start(attnT_sb[:, BL * T:], attnT[1])
        idx_sb = const.tile([S, BL], F32)
        nc.sync.dma_start(idx_sb[:], idx_t[:])
        iota_sb = const.tile([S, VE], F32)
        nc.gpsimd.iota(iota_sb[:], pattern=[[1, VE]], base=0, channel_multiplier=0,
                       allow_small_or_imprecise_dtypes=True)
        # kill ext-vocab column 0 (UNK): make it unmatchable
        nc.gpsimd.memset(iota_sb[:, 0:1], -1.0)

        # ---- copy gate: g1m[t, b] = 1 - sigmoid(dec[t,b] . Wc + bc) ----
        g1m = const.tile([64, BL], F32)
        for b in range(BL):
            gp = ps_ext.tile([64, EXT_N], F32, tag="eps")
            for k in range(KT):
                lhs = dT_sb[:, k * RL + b: k * RL + b + (T - 1) * BL + 1: BL]
                nc.tensor.matmul(gp[:T, 0:1], lhsT=lhs, rhs=wcT_sb[:, k:k + 1],
                                 start=(k == 0), stop=False)
            nc.tensor.matmul(gp[:T, 0:1], lhsT=ones50[:, :T], rhs=bc_sb[:],
                             start=False, stop=True)
            sig = extstage.tile([64, EXT_N], F32, tag="exts")
            nc.scalar.activation(sig[:T, 0:1], gp[:T, 0:1], AF.Sigmoid)
            nc.vector.tensor_scalar(g1m[:T, b:b + 1], sig[:T, 0:1], -1.0, 1.0,
                                    OP.mult, OP.add)

        # ---- ext part emitter (per local batch element) ----
        def emit_ext(b):
            oh = ohpool.tile([S, VE], BF16)
            nc.gpsimd.tensor_scalar(oh[:], iota_sb[:], idx_sb[:, b:b + 1], None,
                                    OP.is_equal)
            for nb in range(EXT_NB):
                ps = ps_ext.tile([64, EXT_N], F32, tag="eps")
                rhs = oh[:, nb * EXT_N:(nb + 1) * EXT_N]
                nc.tensor.matmul(ps[:T, :], lhsT=attnT_sb[:, b * T:(b + 1) * T],
                                 rhs=rhs, start=True, stop=False)
                nc.tensor.matmul(
                    ps[:T, :],
                    lhsT=attnT_sb[:, BL * T + b * T: BL * T + (b + 1) * T],
                    rhs=rhs, start=False, stop=True)
                st = extstage.tile([64, EXT_N], F32, tag="exts")
                # ext = raw * g1m; gate folded into Ln's per-partition scale,
                # clip done in log space (Ln(0) = -inf clips to LOG_LO)
                nc.scalar.activation(st[:T, :], ps[:T, :], AF.Ln,
                                     scale=g1m[:T, b:b + 1])
                nc.vector.tensor_scalar(st[:T, :], st[:T, :], LOG_LO, LOG_HI,
                                        OP.max, OP.min)
                nc.sync.dma_start(out_ext[b, :, nb * EXT_N:(nb + 1) * EXT_N],
                                  st[:T, :])

        # interleave ext batches between main chunks
        ext_sched = {0: [0, 1], 1: [2, 3], 2: [4, 5], 3: [6], 4: [7]}

        # ---- main: logits, online logsumexp, output ----
        for g in range(NCH):
            sums_g = statpool.tile([128, CH], F32, tag="sums")
            x_tiles = []
            for j in range(CH):
                m = g * CH + j
                x_m = xpool.tile([128, VL], BF16, tag="x")
                x_tiles.append(x_m)
                for q in range(NQ):
                    ps = ps_main.tile([128, 1024], F32)
                    for k in range(KT):
                        for nn in range(2):
                            n = 2 * q + nn
                            last = (k == KT - 1) and not with_bias
                            nc.tensor.matmul(
                                ps[:, nn * 512: nn * 512 + NW],
                                lhsT=hT_sb[k][:, m * 128:(m + 1) * 128],
                                rhs=wT_sb[k][:, n * NW:(n + 1) * NW],
                                start=(k == 0), stop=last)
                    if with_bias:
                        for nn in range(2):
                            n = 2 * q + nn
                            nc.tensor.matmul(
                                ps[:, nn * 512: nn * 512 + NW],
                                lhsT=ones_sb[:],
                                rhs=b_sb[:, n * NW:(n + 1) * NW],
                                start=False, stop=True)
                    # psum pair -> x (bf16), one strided copy per pair
                    src = ps[:].rearrange("p (b n) -> p b n", b=2)[:, :, :NW]
                    dst = x_m[:, q * 2 * NW:(q + 1) * 2 * NW].rearrange(
                        "p (b n) -> p b n", b=2)
                    if q % 2 == 0:
                        nc.vector.tensor_copy(dst, src)
                    else:
                        nc.scalar.copy(dst, src)
                E = epool.tile([128, VL], BF16, tag="E")
                nc.scalar.activation(E[:], x_m[:], AF.Exp,
                                     accum_out=sums_g[:, j:j + 1])

            nc.sync.dma_start(cc_in[g][:], sums_g[:])
            nc.gpsimd.collective_compute(
                "AllReduce", OP.add,
                replica_groups=[core_ids],
                ins=[cc_in[g][:]], outs=[cc_out[g][:]])
            tot_g = statpool.tile([128, CH], F32, tag="tot")
            nc.sync.dma_start(tot_g[:], cc_out[g][:])
            neglse = statpool.tile([128, CH], F32, tag="lse")
            nc.scalar.activation(neglse[:], tot_g[:], AF.Ln)
            nc.vector.tensor_scalar(neglse[:], neglse[:], -1.0, None, OP.mult)

            for j in range(CH):
                m = g * CH + j
                x_m = x_tiles[j]
                for half in range(2):
                    o = outpool.tile([128, VL // 2], F32)
                    src = x_m[:, half * (VL // 2):(half + 1) * (VL // 2)]
                    if half == 0:
                        nc.scalar.activation(o[:], src, AF.Identity,
                                             bias=neglse[:, j:j + 1])
                    else:
                        nc.vector.tensor_scalar(o[:], src, neglse[:, j:j + 1],
                                                None, OP.add)
                    nc.sync.dma_start(
                        out_tgt[m * 128:(m + 1) * 128,
                                half * (VL // 2):(half + 1) * (VL // 2)],
                        o[:])

            for b in ext_sched.get(g, []):
                emit_ext(b)

    nc.compile()
    _dedupe_act_table_loads(nc)
    return nc


def _get_nc(with_bias=False):
    key = ("nc", with_bias)
    if key not in _CACHE:
        _CACHE[key] = _build(with_bias)
    return _CACHE[key]


def kernel(**inputs):
    hidden = np.asarray(inputs["hidden"], dtype=np.float32)
    dec = np.asarray(inputs["dec_rnn_output"], dtype=np.float32)
    attn = np.asarray(inputs["attn"], dtype=np.float32)
    c2e = np.asarray(inputs["copy_to_ext"])
    W = np.asarray(inputs["W"], dtype=np.float32)
    bvec = np.asarray(inputs["b"], dtype=np.float32)
    Wc = np.asarray(inputs["Wc"], dtype=np.float32)
    bc = np.asarray(inputs["bc"], dtype=np.float32)

    with_bias = bool(np.any(bvec))
    bf = ml_dtypes.bfloat16
    hT_np = np.ascontiguousarray(
        hidden.reshape(R, H).T.reshape(KT, 128, R)).astype(bf)
    wcT_np = np.ascontiguousarray(Wc.reshape(1, H).T.reshape(KT, 128, 1))
    bc_np = bc.reshape(1, 1)

    in_maps = []
    for c in range(N_CORES):
        vs = slice(c * VL, (c + 1) * VL)
        bs = slice(c * BL, (c + 1) * BL)
        wT_np = np.ascontiguousarray(W[vs].T.reshape(KT, 128, VL)).astype(bf)
        dT_np = np.ascontiguousarray(
            dec[:, bs, :].reshape(RL, H).T.reshape(KT, 128, RL))
        # attnT[s, b*T + t] = attn[t, c*BL+b, s]; hi/lo bf16 split
        at = np.ascontiguousarray(
            attn[:, bs, :].transpose(2, 1, 0).reshape(S, BL * T))
        at_hi = at.astype(bf)
        at_lo = (at - at_hi.astype(np.float32)).astype(bf)
        attnT_np = np.ascontiguousarray(np.stack([at_hi, at_lo]))
        idx_np = np.ascontiguousarray(c2e[:, bs]).astype(np.float32)
        m = {"dT": dT_np, "wcT": wcT_np, "bc": bc_np,
             "attnT": attnT_np, "idx": idx_np}
        for k in range(KT):
            m[f"hT{k}"] = np.ascontiguousarray(hT_np[k])
            m[f"wT{k}"] = np.ascontiguousarray(wT_np[k])
        if with_bias:
            m["brow"] = bvec[vs].reshape(1, VL).astype(bf)
        in_maps.append(m)

    nc = _get_nc(with_bias)
    res = run_bass_kernel_spmd(nc, in_maps, core_ids=list(range(N_CORES)))

    out = np.empty((T, B, VT + VE), dtype=np.float32)
    for c in range(N_CORES):
        r = res.results[c]
        out[:, :, c * VL:(c + 1) * VL] = r["out_tgt"].reshape(T, B, VL)
        out[:, c * BL:(c + 1) * BL, VT:] = r["out_ext"].transpose(1, 0, 2)
    return out
